# revision 57
# baseline (speedup 1.0000x reference)
"""Multi-head attention (B=2, N=2048, D=1024, H=16, HD=64) on 8 TRN2 NeuronCores.

Sharding: core c handles batch b = c//4 and heads 4*(c%4) .. 4*(c%4)+3.
Each core computes the QKV projection for its 4 heads, attention, and a
partial output projection (contraction over its 256 hd-columns of w_out).
The host sums the 4 partial outputs per batch (the tensor-parallel
all-reduce) while unsharding.

v2 design notes (trace-driven):
  - ACT exp is the hard floor: 128 insts x [128,1024] f32-psum -> bf16 at
    ~1.07us each (0.96 GHz, 1 elem/cycle/lane) = ~137us. Everything else is
    scheduled to hide under it; ACT does exp ONLY (copies moved to DVE/gpsimd).
  - All SBUF operands bf16 (halves LDWEIGHTS via FWL, halves input DMA to
    ~16us, halves SBUF traffic). PSUM stays f32; y output f32.
  - Startup: one big wqv DMA + 8 xT chunk DMAs; 8 projection groups run
    chunk-major across 8 PSUM slots so PE tracks DMA arrival (~90% busy
    through the load window, HAM stays warm).
  - Phase 2 per (pair,qc,kb) iter: concurrent score MM pair (rows 0-63 /
    64-127), exp, AV pair (one kb behind). Remaining projection groups,
    v-groups and output-projection pieces are spliced as per-iter filler by
    a static schedule so PE stays ~95% busy at the exp cadence.
  - Pair 1 processes qc in reverse (3..0) so each qc's outproj can be
    spread over the following 16 iters; the last qc's (qc0) outproj is
    split per head-pair: the c2=0 half runs early (iters 16..30, into SBUF
    partials), only the c2=1 half + DVE adds remain in the tail.
  - Softmax denominator: ones-column in v (row 64 of the AV psum). Scale
    chain per pair: copy po->oacc (frees psum), 1-lane reciprocal of the
    den row, DRAM-bounce broadcast to [64,512], one mul per head
    (vector/gpsimd alternate).
"""

import os
import sys
import types
import ctypes
import contextlib

import numpy as np
import ml_dtypes
import bass_rust
import concourse.bass as bass
import concourse.tile as tile
from concourse import mybir
from concourse import bass_utils
from concourse.vector_clock import ScopedClock


def _ensure_ntff_hook():
    """Provide antenv.axon_hooks if the container lacks it, so that
    run_bass_kernel_spmd(trace=True) (e.g. via BASS_TRACE=1) works instead
    of raising ModuleNotFoundError."""
    if "antenv.axon_hooks" in sys.modules:
        return
    try:
        import antenv.axon_hooks  # noqa: F401

        return
    except ImportError:
        pass

    def _make_hook():
        so_path = "/opt/axon/libaxon_pjrt.so"
        try:
            lib = ctypes.CDLL(so_path)
        except OSError:
            return None
        if not hasattr(lib, "axon_start_nrt_profile"):
            return None
        lib.axon_start_nrt_profile.argtypes = [
            ctypes.POINTER(ctypes.c_int64),
            ctypes.c_size_t,
        ]
        lib.axon_start_nrt_profile.restype = ctypes.c_int64
        lib.axon_stop_nrt_profile.argtypes = [ctypes.c_char_p]
        lib.axon_stop_nrt_profile.restype = ctypes.c_int64

        @contextlib.contextmanager
        def _hook(output_dir, device_ids):
            import jax

            jax.devices()
            if device_ids:
                ids = (ctypes.c_int64 * len(device_ids))(*device_ids)
                rc = lib.axon_start_nrt_profile(ids, len(device_ids))
            else:
                rc = lib.axon_start_nrt_profile(None, 0)
            if rc != 0:
                raise RuntimeError(f"axon_start_nrt_profile rc={rc}")
            try:
                yield
            finally:
                lib.axon_stop_nrt_profile(str(output_dir).encode())

        return _hook

    hook = _make_hook()
    mod = types.ModuleType("antenv.axon_hooks")
    mod.get_axon_ntff_profile_hook = lambda: hook
    mod.set_axon_ntff_profile_hook = lambda h: None
    sys.modules["antenv.axon_hooks"] = mod


_ensure_ntff_hook()

B, N, D = 2, 2048, 1024
H, HD = 16, 64
HPG = 4  # heads per core
NCORES = 8
ND = D // 128  # 8 contraction chunks for the projections
NT = N // 128  # 16 token/key blocks
NQ = N // 512  # 4 query chunks

f32 = mybir.dt.float32
bf16 = mybir.dt.bfloat16
EXP = mybir.ActivationFunctionType.Exp


class _TC(tile.TileContext):
    """TileContext adapted to this walrus build, which encodes at most ONE
    semaphore wait per instruction: excess waits are offloaded onto
    preceding same-engine nops, and the final drain is split the same way."""

    _ws_counter = 0

    def _lower_ordered_insts(self, ordered):
        for bbname, insts in ordered.items():
            new = []
            for inst in insts:
                si = inst.sync_info
                if (
                    si is not None
                    and len(si.on_wait) > 1
                    and inst.engine != mybir.EngineType.Unassigned
                ):
                    waits = list(si.on_wait)
                    ups = list(si.on_update)
                    for w in waits[:-1]:
                        _TC._ws_counter += 1
                        new.append(
                            mybir.InstNoOp(
                                name=f"waitsplit_{_TC._ws_counter}",
                                engine=inst.engine,
                                ins=[],
                                outs=[],
                                sync_info=bass_rust.SyncInfo(
                                    on_wait=[w], on_update=[]
                                ),
                                bass_nofuse=True,
                            )
                        )
                    inst.sync_info = bass_rust.SyncInfo(
                        on_wait=[waits[-1]], on_update=ups
                    )
                new.append(inst)
            ordered[bbname] = new
        super()._lower_ordered_insts(ordered)

    def _drain_and_barrier(self, tick_clock, wait_clock):
        nop0 = self.nc.sync.nop(nofuse=True)
        wait_clock.add_sem_waits(nop0.ins, ScopedClock({None: tick_clock.global_clock}))
        si = nop0.ins.sync_info
        waits = list(si.on_wait) if si is not None else []
        if len(waits) > 1:
            nop0.ins.sync_info = bass_rust.SyncInfo(on_wait=waits[:1], on_update=[])
            for i in range(1, len(waits)):
                n = self.nc.sync.nop(nofuse=True)
                n.ins.sync_info = bass_rust.SyncInfo(
                    on_wait=waits[i : i + 1], on_update=[]
                )
        self.nc.sync.drain()
        self.nc.all_engine_barrier()
        assert self.sems is not None
        popped = self.nc._tile_sem_poison_stack.pop()
        assert popped is self._sem_poison
        self.nc.clear_and_free_semaphores(list(self.sems.allocated().values()))
        self.nc.all_engine_barrier()


def _body(nc, tc, xT, wqv, wo, y):
    with contextlib.ExitStack() as ctx:
        persist = ctx.enter_context(tc.tile_pool(name="persist", bufs=1))
        pt_pool = ctx.enter_context(tc.tile_pool(name="ptp", bufs=4))
        ysb_pool = ctx.enter_context(tc.tile_pool(name="ysbp", bufs=4))
        small = ctx.enter_context(tc.tile_pool(name="small", bufs=4))
        dscr = ctx.enter_context(tc.tile_pool(name="dscr", bufs=4, space="DRAM"))
        ps_s = ctx.enter_context(tc.tile_pool(name="ps_s", bufs=2, space="PSUM"))
        ps_o = ctx.enter_context(tc.tile_pool(name="ps_o", bufs=2, space="PSUM"))
        ps_mm = ctx.enter_context(tc.tile_pool(name="ps_mm", bufs=2, space="PSUM"))

        # ---- persistent SBUF residents ----
        xT_big = persist.tile([128, ND * N], bf16, tag="xT", name="xT_big")
        wqv_big = persist.tile([128, ND * 768], bf16, tag="wqv", name="wqv_big")
        wo_big = persist.tile([128, 2 * D], bf16, tag="wo", name="wo_big")

        def xc(i):
            return xT_big[:, i * N : (i + 1) * N]

        def wqk_c(i):
            return wqv_big[:, i * 768 : i * 768 + 512]

        def wv_c(i):
            return wqv_big[:, i * 768 + 512 : (i + 1) * 768]

        # Input DMA: per-chunk weight+x pairs so the chunk-major prefix wave
        # tracks arrival; wo last (first needed ~iter 24).
        wqv_dram = wqv.rearrange("(c p) m -> p c m", p=128)
        for i in range(ND):
            nc.sync.dma_start(
                out=wqv_big[:, i * 768 : (i + 1) * 768], in_=wqv_dram[:, i, :]
            )
            nc.sync.dma_start(out=xc(i), in_=xT[i * 128 : (i + 1) * 128, :])
        nc.sync.dma_start(
            out=wo_big.rearrange("p (c m) -> p c m", c=2),
            in_=wo.rearrange("(c p) m -> p c m", p=128),
        )

        # qkT rows: tile 0 = qT heads 0,1 | tile 1 = qT heads 2,3
        #           tile 2 = kT heads 0,1 | tile 3 = kT heads 2,3
        qkT_sb = [
            persist.tile([128, N], bf16, tag=f"qkT{r}", name=f"qkT_sb{r}")
            for r in range(4)
        ]
        # v blocks with a ones column after each head: [v_h | 1] x 4
        v_sb = [
            persist.tile([128, HPG * (HD + 1)], bf16, tag=f"v{t}", name=f"v_sb{t}")
            for t in range(NT)
        ]
        oT_sb = [
            persist.tile([128, N], bf16, tag=f"oT{c2}", name=f"oT_sb{c2}")
            for c2 in range(2)
        ]
        # SBUF partials for the split output projection: every piece's c2=0
        # half (pair-0 heads) runs early as filler; the c2=1 half + add
        # trails that qc's pair-1 den chain. 32 pieces = qc*8 + tb*2 + dc.
        ysb0 = [
            persist.tile([128, 512], f32, tag=f"ysb0_{j}", name=f"ysb0_{j}")
            for j in range(32)
        ]
        for t in range(NT):
            nc.vector.memset(v_sb[t], 1.0)
        ident = persist.tile([128, 1], f32, tag="ident", name="ident")
        nc.vector.memset(ident, 1.0)

        # ---- group emitters ----
        def qk_mm(r, qc, ps, i):
            nc.tensor.matmul(
                ps,
                lhsT=wqk_c(i)[:, r * 128 : (r + 1) * 128],
                rhs=xc(i)[:, qc * 512 : (qc + 1) * 512],
                start=(i == 0),
                stop=(i == ND - 1),
            )

        def qk_finish(r, qc, ps, eng="vector"):
            dst = qkT_sb[r][:, qc * 512 : (qc + 1) * 512]
            if eng == "scalar":
                nc.scalar.copy(dst, ps)
            else:
                nc.vector.tensor_copy(dst, ps)

        def v_mm(t, ps, i):
            nc.tensor.matmul(
                ps,
                lhsT=xc(i)[:, t * 128 : (t + 1) * 128],
                rhs=wv_c(i),
                start=(i == 0),
                stop=(i == ND - 1),
            )

        def v_finish(t, ps):
            vview = v_sb[t].rearrange("p (h c) -> p h c", c=HD + 1)[:, :, 0:HD]
            nc.vector.tensor_copy(vview, ps.rearrange("p (h c) -> p h c", c=HD))

        def qk_group_unit(r, qc):
            """Split into 4 steps of 2 chained MMs + a finish step, so the
            per-iter filler stays under the exp cadence."""
            box = {}

            def mk(i0):
                def step():
                    if "ps" not in box:
                        box["ps"] = ps_mm.tile(
                            [128, 512], f32, tag="mm", name=f"ps_qk_{r}_{qc}"
                        )
                    qk_mm(r, qc, box["ps"], i0)
                    qk_mm(r, qc, box["ps"], i0 + 1)

                return step

            def fin():
                qk_finish(r, qc, box["ps"])

            return [(mk(i0), 426) for i0 in range(0, ND, 2)] + [(fin, 50)]

        def v_group_unit(t):
            box = {}

            def mk(i0):
                def step():
                    if "ps" not in box:
                        box["ps"] = ps_mm.tile(
                            [128, HPG * HD], f32, tag="mm", name=f"ps_v_{t}"
                        )
                    for i in range(i0, i0 + 4):
                        v_mm(t, box["ps"], i)

                return step

            def fin():
                v_finish(t, box["ps"])

            return [(mk(0), 428), (mk(4), 428), (fin, 50)]

        # ---- output projection halves (t = global token block 0..15) ----
        def outproj_half0_unit(t, dc, j):
            # c2=0 half (pair-0 heads) -> SBUF partial ysb0[j]
            def emit():
                ps = ps_mm.tile([128, 512], f32, tag="mm", name=f"ps_y0_{t}_{dc}")
                nc.tensor.matmul(
                    ps,
                    lhsT=oT_sb[0][:, t * 128 : (t + 1) * 128],
                    rhs=wo_big[:, dc * 512 : (dc + 1) * 512],
                    start=True,
                    stop=True,
                )
                nc.vector.tensor_copy(ysb0[j], ps)

            return [(emit, 250)]

        def outproj_half1(t, dc, j):
            # c2=1 half (pair-1 heads) + add partial + store
            ps = ps_mm.tile([128, 512], f32, tag="mm", name=f"ps_y1_{t}_{dc}")
            nc.tensor.matmul(
                ps,
                lhsT=oT_sb[1][:, t * 128 : (t + 1) * 128],
                rhs=wo_big[:, D + dc * 512 : D + (dc + 1) * 512],
                start=True,
                stop=True,
            )
            ysb = ysb_pool.tile([128, 512], f32, tag="y", name=f"ysb1_{t}_{dc}")
            nc.vector.tensor_add(ysb, ysb0[j], ps)
            nc.sync.dma_start(
                out=y[t * 128 : (t + 1) * 128, dc * 512 : (dc + 1) * 512],
                in_=ysb,
            )

        def outproj_half1_unit(t, dc, j):
            return [(lambda: outproj_half1(t, dc, j), 300)]

        # ---- filler unit queue: (ready_iter, deadline_iter, steps) ----
        # Units drain strictly in order; an in-flight unit finishes all its
        # steps before the next starts (keeps the 2-deep "mm" PSUM ring
        # hazard-free). Per iter the drainer emits up to ~380ns of PE work,
        # or more when a deadline is due.
        units = []
        # All projection units are data-ready from the start (inputs land in
        # the prefix); ORDER + staggered deadlines pace them. The strict
        # front-first drainer means a not-yet-ready unit blocks everything
        # behind it, so ready-gated units (outproj) go last.
        # v3..13 then qk(0,1) then v14,v15: the qk unit's deadline (14)
        # interleaves it into the late v-band instead of bursting at 13-14
        for t in range(3, 14):
            units.append((0, t, v_group_unit(t)))
        units.append((0, 14, qk_group_unit(0, 1)))
        units.append((0, 14, v_group_unit(14)))
        units.append((0, 15, v_group_unit(15)))
        units.append((0, 28, qk_group_unit(0, 2)))
        units.append((0, 34, qk_group_unit(1, 0)))
        units.append((0, 38, qk_group_unit(3, 0)))
        units.append((0, 44, qk_group_unit(0, 3)))
        units.append((0, 46, qk_group_unit(1, 1)))
        units.append((0, 50, qk_group_unit(3, 1)))
        units.append((0, 52, qk_group_unit(1, 2)))
        units.append((0, 54, qk_group_unit(3, 2)))
        units.append((0, 56, qk_group_unit(1, 3)))
        units.append((0, 60, qk_group_unit(3, 3)))
        # c2=0 halves: pair-0 oT for qc is scaled ~11 iters after its pair-0
        # block ((0,qc) ends at iter 16qc+15)
        for j in range(8):
            units.append((29 + j, 100, outproj_half0_unit(j // 2, j % 2, j)))
        for j in range(8):
            units.append((44 + j, 104, outproj_half0_unit(4 + j // 2, j % 2, 8 + j)))
        for j in range(8):
            units.append((60 + j, 108, outproj_half0_unit(8 + j // 2, j % 2, 16 + j)))
        for j in range(8):
            units.append((76 + j, 112, outproj_half0_unit(12 + j // 2, j % 2, 24 + j)))
        # c2=1 halves trail each pair-1 den chain (blocks at 64-79/80-95/
        # 96-111 for qc3/qc2/qc1; chain adds ~11.5 iters)
        for j in range(8):
            units.append((93 + j // 2, 126, outproj_half1_unit(12 + j // 2, j % 2, 24 + j)))
        for j in range(8):
            units.append((109 + j // 2, 126, outproj_half1_unit(8 + j // 2, j % 2, 16 + j)))
        for j in range(8):
            units.append((124 + j // 4, 127, outproj_half1_unit(4 + j // 2, j % 2, 8 + j)))

        uq = list(units)  # consumed front-first
        cur = {"steps": [], "deadline": 10**9}

        def drain_filler(git):
            budget = 380.0
            while True:
                if not cur["steps"]:
                    if not uq or uq[0][0] > git:
                        return
                    if budget <= 0 and uq[0][1] > git + 2:
                        return
                    ready, deadline, steps = uq.pop(0)
                    cur["steps"] = list(steps)
                    cur["deadline"] = deadline
                fn, cost = cur["steps"].pop(0)
                fn()
                budget -= cost
                if budget <= 0 and cur["steps"] and cur["deadline"] > git + 2:
                    return

        # ---- prefix: 8 groups chunk-major across 8 PSUM slots ----
        s1 = ps_s.tile([128, 1024], f32, tag="s", name="pfx_s1")
        s2 = ps_s.tile([128, 1024], f32, tag="s", name="pfx_s2")
        m1 = ps_mm.tile([128, 512], f32, tag="mm", name="pfx_m1")
        m2 = ps_mm.tile([128, 512], f32, tag="mm", name="pfx_m2")
        o1 = ps_o.tile([128, 512], f32, tag="o", name="pfx_o1")
        o2 = ps_o.tile([128, 512], f32, tag="o", name="pfx_o2")
        pfx = [
            (lambda ps, i: qk_mm(2, 0, ps, i), lambda ps: qk_finish(2, 0, ps, "scalar"), s1[:, 0:512]),
            (lambda ps, i: qk_mm(0, 0, ps, i), lambda ps: qk_finish(0, 0, ps, "scalar"), m1),
            (lambda ps, i: qk_mm(2, 1, ps, i), lambda ps: qk_finish(2, 1, ps), s1[:, 512:1024]),
            (lambda ps, i: qk_mm(2, 2, ps, i), lambda ps: qk_finish(2, 2, ps), s2[:, 0:512]),
            (lambda ps, i: qk_mm(2, 3, ps, i), lambda ps: qk_finish(2, 3, ps), s2[:, 512:1024]),
            (lambda ps, i: v_mm(0, ps, i), lambda ps: v_finish(0, ps), m2[:, 0:256]),
            (lambda ps, i: v_mm(1, ps, i), lambda ps: v_finish(1, ps), o1[:, 0:256]),
            (lambda ps, i: v_mm(2, ps, i), lambda ps: v_finish(2, ps), o2[:, 0:256]),
        ]
        for i in range(ND):
            for mm, _fin, ps in pfx:
                mm(ps, i)
        for _mm, fin, ps in pfx:
            fin(ps)

        # ---- phase 2: attention + interleaved filler ----
        def av_pair(pair, poA, poB, kb, pt):
            hA, hB = 2 * pair, 2 * pair + 1
            nc.tensor.matmul(
                poA,
                lhsT=v_sb[kb][:, hA * (HD + 1) : (hA + 1) * (HD + 1)],
                rhs=pt[:, 0:512],
                start=(kb == 0),
                stop=(kb == NT - 1),
            )
            nc.tensor.matmul(
                poB,
                lhsT=v_sb[kb][:, hB * (HD + 1) : (hB + 1) * (HD + 1)],
                rhs=pt[:, 512:1024],
                start=(kb == 0),
                stop=(kb == NT - 1),
            )

        def scale_pair_part1(pair, qc, poA, poB):
            # Softmax denominators live in psum row 64 (the v ones-column).
            # Copy each po to SBUF (frees the psum bank fast) and launch the
            # DRAM bounce that reshapes the den row [1,512]->[128,4] so the
            # reciprocal can use all DVE lanes. Returns the part-2 closure,
            # which the caller emits a couple of iterations later (when the
            # DMAs have landed) so the in-order vector queue never blocks.
            parts = []
            for h, po in ((2 * pair, poA), (2 * pair + 1, poB)):
                qb = (h % 2) * 64
                oacc = small.tile(
                    [65, 512], f32, tag="oacc", name=f"oacc_{qc}_{h}", bufs=4
                )
                nc.vector.tensor_copy(oacc, po)
                # den row -> DRAM -> [128,4] (an SBUF->SBUF repartition DMA
                # returns garbage on HW even though CoreSim accepts it)
                scr = dscr.tile([1, 512], f32, tag="scr", name=f"scr_{qc}_{h}")
                nc.sync.dma_start(out=scr, in_=oacc[64:65, :])
                rin = small.tile([128, 4], f32, tag="rin", name=f"rin_{qc}_{h}", bufs=4)
                nc.sync.dma_start(
                    out=rin, in_=scr.rearrange("a (p c) -> (a p) c", c=4)
                )
                parts.append((h, qb, oacc, rin))

            reps = []

            def part2a():
                # reciprocal (rin landed by now) + broadcast DMAs
                for h, qb, oacc, rin in parts:
                    rout = small.tile(
                        [128, 4], f32, tag="rout", name=f"rout_{qc}_{h}", bufs=4
                    )
                    nc.vector.reciprocal(rout, rin)
                    scr2 = dscr.tile([1, 512], f32, tag="scr2", name=f"scr2_{qc}_{h}")
                    nc.sync.dma_start(
                        out=scr2.rearrange("a (p c) -> (a p) c", c=4), in_=rout
                    )
                    rep = small.tile(
                        [64, 512], f32, tag="rep", name=f"rep_{qc}_{h}", bufs=4
                    )
                    nc.sync.dma_start(out=rep, in_=scr2.to_broadcast((64, 512)))
                    reps.append(rep)

            def part2b():
                # the scale muls, emitted ~6 iters after part2a so the rep
                # broadcast has landed and the strict-FIFO vector queue never
                # blocks on it
                for (h, qb, oacc, rin), rep in zip(parts, reps):
                    nc.vector.tensor_mul(
                        oT_sb[pair][qb : qb + 64, qc * 512 : (qc + 1) * 512],
                        oacc[0:64, :],
                        rep,
                    )

            return part2a, part2b

        seq = [(0, qc) for qc in range(NQ)] + [(1, qc) for qc in (3, 2, 1, 0)]
        git = 0  # global phase-2 iteration index
        pending_part2a = pending_part2b = None
        for seq_idx, (pair, qc) in enumerate(seq):
            poA = ps_o.tile([65, 512], f32, tag="o", name=f"poA_{pair}_{qc}")
            poB = ps_o.tile([65, 512], f32, tag="o", name=f"poB_{pair}_{qc}")
            pending = None
            for kb in range(NT):
                if kb == 5 and pending_part2a is not None:
                    pending_part2a()
                    pending_part2a = None
                if kb == 11 and pending_part2b is not None:
                    pending_part2b()
                    pending_part2b = None
                ps = ps_s.tile([128, 1024], f32, tag="s", name=f"ps_s_{pair}_{qc}_{kb}")
                nc.tensor.matmul(
                    ps[:, 0:512],
                    lhsT=qkT_sb[2 + pair][0:64, kb * 128 : (kb + 1) * 128],
                    rhs=qkT_sb[pair][0:64, qc * 512 : (qc + 1) * 512],
                    start=True,
                    stop=True,
                )
                nc.tensor.matmul(
                    ps[:, 512:1024],
                    lhsT=qkT_sb[2 + pair][64:128, kb * 128 : (kb + 1) * 128],
                    rhs=qkT_sb[pair][64:128, qc * 512 : (qc + 1) * 512],
                    start=True,
                    stop=True,
                )
                pt = pt_pool.tile(
                    [128, 1024], bf16, tag="pt", name=f"pt_{pair}_{qc}_{kb}"
                )
                nc.scalar.activation(pt, ps, EXP, scale=HD**-0.5)
                if pending is not None:
                    av_pair(pair, poA, poB, *pending)
                pending = (kb, pt)
                drain_filler(git)
                git += 1
            av_pair(pair, poA, poB, *pending)
            if seq_idx < len(seq) - 1:
                pending_part2a, pending_part2b = scale_pair_part1(pair, qc, poA, poB)
        while uq or cur["steps"]:
            drain_filler(10**9)

        # Keep-warm matmuls: the final den chain idles PE ~5us which would
        # re-throttle HAM and double the half1 matmul durations. Dead
        # accumulations into a scratch psum keep K=8/8.
        # Tail: the final block's den chain runs on a fast path - PE
        # transposes lift the den rows onto 128 partitions (skipping the
        # DRAM reshape bounce), and the broadcast DMAs issue from the idle
        # ACT queue so they don't wait behind the y-write descriptors on
        # sync. Keep-warm matmuls cover the remaining ~6us so HAM doesn't
        # re-throttle before the half1 matmuls.
        tail_muls = []
        for h, po in ((2, poA), (2 + 1, poB)):
            qb = (h % 2) * 64
            oacc = small.tile([65, 512], f32, tag="oacc", name=f"oacc_t_{h}", bufs=4)
            nc.vector.tensor_copy(oacc, po)
            tp = ps_mm.tile([128, 4], f32, tag="mm", name=f"tp_{h}")
            for q in range(4):
                nc.tensor.transpose(
                    tp[:, q : q + 1],
                    oacc[64:65, q * 128 : (q + 1) * 128],
                    ident[64:65, 0:1],
                )
            rout = small.tile([128, 4], f32, tag="rout", name=f"rout_t_{h}", bufs=4)
            nc.vector.reciprocal(rout, tp)
            scr2 = dscr.tile([1, 512], f32, tag="scr2", name=f"scr2_t_{h}")
            # transpose layout: rout[p, q] = 1/den[q*128 + p]
            nc.sync.dma_start(
                out=scr2.rearrange("a (c p) -> (a p) c", p=128), in_=rout
            )
            rep = small.tile([64, 512], f32, tag="rep", name=f"rep_t_{h}", bufs=4)
            nc.sync.dma_start(out=rep, in_=scr2.to_broadcast((64, 512)))
            tail_muls.append(
                lambda qb=qb, oacc=oacc, rep=rep: nc.vector.tensor_mul(
                    oT_sb[1][qb : qb + 64, 0:512], oacc[0:64, :], rep
                )
            )
        warm = ps_s.tile([128, 1024], f32, tag="s", name="warm_ps")
        for w in range(25):
            nc.tensor.matmul(
                warm[:, 0:512],
                lhsT=qkT_sb[0][:, 0:128],
                rhs=qkT_sb[1][:, 0:512],
                start=(w == 0),
                stop=(w == 24),
            )
        for m in tail_muls:
            m()
        # tail: qc0 outproj c2=1 halves + partial adds
        for j in range(8):
            outproj_half1(j // 2, j % 2, j)


def build():
    nc = bass.Bass("TRN2", target_bir_lowering=False)
    xT = nc.dram_tensor("xT", [D, N], bf16, kind="ExternalInput").ap()
    wqv = nc.dram_tensor("wqv", [D, 768], bf16, kind="ExternalInput").ap()
    wo = nc.dram_tensor("wo", [HPG * HD, D], bf16, kind="ExternalInput").ap()
    y = nc.dram_tensor("y", [N, D], f32, kind="ExternalOutput").ap()
    with _TC(nc) as tc:
        _body(nc, tc, xT, wqv, wo, y)
    return nc


def shard_inputs(x, w_qkv, w_out):
    """Build the 8 per-core input maps from the full tensors (bf16)."""
    x = np.asarray(x, dtype=np.float32)
    w_qkv = np.asarray(w_qkv, dtype=np.float32)
    w_out = np.asarray(w_out, dtype=np.float32)
    in_maps = []
    for c in range(NCORES):
        b, grp = c // 4, c % 4
        heads = [HPG * grp + i for i in range(HPG)]
        xTa = np.ascontiguousarray(x[b].T).astype(ml_dtypes.bfloat16)
        qcols = [w_qkv[:, h * HD : (h + 1) * HD] for h in heads]
        kcols = [w_qkv[:, H * HD + h * HD : H * HD + (h + 1) * HD] for h in heads]
        vcols = [w_qkv[:, 2 * H * HD + h * HD : 2 * H * HD + (h + 1) * HD] for h in heads]
        wqv_a = np.ascontiguousarray(
            np.concatenate(qcols + kcols + vcols, axis=1)
        ).astype(ml_dtypes.bfloat16)
        wo_a = np.ascontiguousarray(
            np.concatenate([w_out[h * HD : (h + 1) * HD, :] for h in heads], axis=0)
        ).astype(ml_dtypes.bfloat16)
        in_maps.append({"xT": xTa, "wqv": wqv_a, "wo": wo_a})
    return in_maps


LAST_RESULTS = None  # BassKernelResults from the most recent kernel() call
_NC_CACHE = None


def kernel(x, w_qkv, w_out):
    global LAST_RESULTS, _NC_CACHE
    if _NC_CACHE is None:
        _NC_CACHE = build()
    nc = _NC_CACHE
    in_maps = shard_inputs(x, w_qkv, w_out)
    trace = bool(os.environ.get("KERNEL_TRACE"))
    res = bass_utils.run_bass_kernel_spmd(
        nc, in_maps, core_ids=list(range(NCORES)), trace=trace
    )
    LAST_RESULTS = res
    y = np.zeros((B, N, D), dtype=np.float32)
    for c in range(NCORES):
        y[c // 4] += res.results[c]["y"]
    return y
